# revision 2
# baseline (speedup 1.0000x reference)
"""Trainium2 Bass kernel for the EntropyResidualBlock — Winograd variant.

conv1's kh=0,1 tap rows (10 of 13 taps) run as Winograd F(4,5) along W:
B^T input transform on GpSimd (from the bf16 x ring), 8 point-matmuls per
(kh,cic,coc) batched over 2 output rows (N=256, bf16 U/V), A^T output
combine on Vector accumulating into the kh=2 direct-conv PSUM. conv1 kh=2
taps: (2,0),(2,1) cic01 as fp8e4 DoubleRow, rest bf16. conv2 is the
baseline direct conv (5 fp8-DR taps + bf16). Offline sim rel err 1.755e-2.

Sharding: 8 cores = 2 batches x 4 H-strips of 64 rows (as baseline), with a
2-row recomputed y1 halo masked by hm for top strips.
"""

import os
import sys

import numpy as np
import ml_dtypes

for _p in ("/opt/trn_rl_repo",):
    if os.path.isdir(_p) and _p not in sys.path:
        sys.path.append(_p)

import concourse.bass as bass  # noqa: E402
import concourse.tile as tile  # noqa: E402
from concourse import bacc, mybir  # noqa: E402
from concourse.bass import ds  # noqa: E402
from concourse.bass_utils import run_bass_kernel_spmd  # noqa: E402

BF16NP = ml_dtypes.bfloat16
E4NP = ml_dtypes.float8_e4m3
F32 = mybir.dt.float32
BF16 = mybir.dt.bfloat16
FP8 = mybir.dt.float8e4
AF = mybir.ActivationFunctionType
DRM = mybir.MatmulPerfMode.DoubleRow
ALU = mybir.AluOpType

B, C, H, W = 2, 384, 256, 512
NG, CPN, KS, PAD = 16, 24, 5, 2
NCORES = 8
SPB = 4            # strips per batch
HS = H // SPB      # 64 output rows per core
WP = 520           # padded row width (2 left + 512 + 6 right, all zeros)
WP8 = 528          # fp8 ring row pitch
NR = HS + 5        # x rows staged per core
NT2 = 13           # conv2 direct taps
TAPS2 = [(kh, kw) for kh in (0, 1) for kw in range(KS)] + [(2, 0), (2, 1), (2, 2)]
FP8_TAPS2 = [(0, 0), (0, 2), (0, 4), (1, 1), (1, 3)]
NF2 = len(FP8_TAPS2)
DTAPS1 = [(2, 0), (2, 1), (2, 2)]      # conv1 direct taps
FP8_TAPS1 = [(2, 0), (2, 1)]
SW, SX = 1024.0, 16.0
SB = SW * SX
NPAIRS = HS // 2


def _wino_mats():
    """F(4,5) Cook-Toom at points [0,1,-1,2,-2,1/2,-1/2,inf]; exact (verified
    to 1e-13 by construction in the offline sim)."""
    m, R = 4, 5
    pts = [0, 1, -1, 2, -2, 0.5, -0.5]
    n = m + R - 1
    a = np.array(pts, np.float64)
    At = np.zeros((m, n))
    for i in range(m):
        At[i, :n - 1] = a ** i
    At[m - 1, n - 1] = 1.0
    G = np.zeros((n, R))
    for i in range(n - 1):
        Ni = np.prod([a[i] - a[k] for k in range(n - 1) if k != i])
        G[i] = (a[i] ** np.arange(R)) / Ni
    G[n - 1, R - 1] = 1.0
    M = np.zeros((m * R, n))
    for k in range(m):
        for j in range(R):
            M[k * R + j] = At[k] * G[:, j]
    Bt = np.zeros((n, n))
    for l in range(n):
        c = np.zeros(m * R)
        for k in range(m):
            for j in range(R):
                c[k * R + j] = 1.0 if l == k + j else 0.0
        Bt[:, l] = np.linalg.lstsq(M, c, rcond=None)[0]
    return At, G, Bt


_AT, _G, _BT = _wino_mats()


def _build_mask() -> np.ndarray:
    m = np.zeros((C, C, KS, KS), np.float32)
    m[:, :, :PAD, :] = 1.0
    m[:, :, PAD, :PAD] = 1.0
    g = np.arange(C) // CPN
    m[:, :, PAD, PAD] = (g[None, :] <= g[:, None]).astype(np.float32)
    return m


def _build_nc():
    nc = bacc.Bacc("TRN2", target_bir_lowering=False, debug=False,
                   num_devices=NCORES)
    xs_d = nc.dram_tensor("xs", [128, NR * 3, WP], BF16, kind="ExternalInput").ap()
    u1_d = nc.dram_tensor("u1t", [128, 3, 2, 8, 3, 128], BF16,
                          kind="ExternalInput").ap()
    w1d_d = nc.dram_tensor("w1d", [128, 3, 3, 3, 128], BF16,
                           kind="ExternalInput").ap()
    w18_d = nc.dram_tensor("w18", [128, 3, 2, 2, 128], FP8,
                           kind="ExternalInput").ap()
    w2_d = nc.dram_tensor("w2t", [128, 3, NT2, 3, 128], BF16,
                          kind="ExternalInput").ap()
    w28_d = nc.dram_tensor("w28", [128, 3, NF2, 2, 128], FP8,
                           kind="ExternalInput").ap()
    b1_d = nc.dram_tensor("b1c", [128, 3], F32, kind="ExternalInput").ap()
    a1_d = nc.dram_tensor("a1c", [128, 3], F32, kind="ExternalInput").ap()
    b2_d = nc.dram_tensor("b2c", [128, 3], F32, kind="ExternalInput").ap()
    a2_d = nc.dram_tensor("a2c", [128, 3], F32, kind="ExternalInput").ap()
    b116_d = nc.dram_tensor("b1c16", [128, 3], F32, kind="ExternalInput").ap()
    hm_d = nc.dram_tensor("hm", [128, 2], F32, kind="ExternalInput").ap()
    ys_d = nc.dram_tensor("ys", [128, (HS + 2) * 3, W], BF16,
                          kind="ExternalOutput").ap()

    with tile.TileContext(nc) as tc:
        with tc.tile_pool(name="wp", bufs=1) as wp, \
             tc.tile_pool(name="cp", bufs=1) as cp, \
             tc.tile_pool(name="ring", bufs=1) as rp, \
             tc.tile_pool(name="tmp", bufs=3) as tp, \
             tc.tile_pool(name="op", bufs=4) as op, \
             tc.tile_pool(name="ppw", bufs=1, space="PSUM") as ppw, \
             tc.tile_pool(name="ppd", bufs=4, space="PSUM") as ppd:

            w18 = wp.tile([128, 3, 2, 2, 128], FP8, name="w18sb", tag="w18sb")
            nc.gpsimd.dma_start(out=w18, in_=w18_d)
            u1t = wp.tile([128, 3, 2, 8, 3, 128], BF16, name="u1sb", tag="u1sb")
            for _c in range(3):
                nc.gpsimd.dma_start(out=u1t[:, _c], in_=u1_d[:, _c])
            w1dt = wp.tile([128, 3, 3, 3, 128], BF16, name="w1dsb", tag="w1dsb")
            nc.gpsimd.dma_start(out=w1dt, in_=w1d_d)
            b1c = cp.tile([128, 3], F32, name="b1sb", tag="b1sb")
            nc.gpsimd.dma_start(out=b1c, in_=b1_d)
            a1c = cp.tile([128, 3], F32, name="a1sb", tag="a1sb")
            nc.gpsimd.dma_start(out=a1c, in_=a1_d)
            b2c = cp.tile([128, 3], F32, name="b2sb", tag="b2sb")
            nc.gpsimd.dma_start(out=b2c, in_=b2_d)
            a2c = cp.tile([128, 3], F32, name="a2sb", tag="a2sb")
            nc.gpsimd.dma_start(out=a2c, in_=a2_d)
            b1c16 = cp.tile([128, 3], F32, name="b116sb", tag="b116sb")
            nc.gpsimd.dma_start(out=b1c16, in_=b116_d)
            hm = cp.tile([128, 2], F32, name="hmsb", tag="hmsb")
            nc.gpsimd.dma_start(out=hm, in_=hm_d)

            # rings: xb keyed s%6 (s = x row + 4), x8 and y-rings keyed %4,
            # V keyed s%4 with slot 4 = shadow of slot 0
            xb = [rp.tile([128, 3, WP], BF16, name=f"xb{j}", tag=f"xb{j}")
                  for j in range(6)]
            x8 = [rp.tile([128, 2, WP8], FP8, name=f"x8{j}", tag=f"x8{j}")
                  for j in range(4)]
            vt = rp.tile([128, 5, 3, 8, 128], BF16, name="vt", tag="vt")
            y1 = [rp.tile([128, 3, WP], BF16, name=f"y1r{j}", tag=f"y1r{j}")
                  for j in range(4)]
            y18 = [rp.tile([128, 2, WP8], FP8, name=f"y18r{j}", tag=f"y18r{j}")
                   for j in range(4)]
            for j in range(4):
                nc.vector.memset(y1[j][:, :, 0:PAD], 0.0)
                nc.vector.memset(y1[j][:, :, PAD + W:WP], 0.0)
                nc.vector.memset(y18[j][:, :, 0:PAD], 0.0)
                nc.vector.memset(y18[j][:, :, PAD + W:WP8], 0.0)

            def load_x_row(s):
                # xs row s (= x row s-4): one DMA
                nc.sync.dma_start(out=xb[s % 6], in_=xs_d[:, ds(s * 3, 3), :])

            def cast_x8(s):
                nc.scalar.activation(x8[s % 4][:, :, 0:WP],
                                     xb[s % 6][:, 0:2, :], AF.Copy, scale=SX)

            def bt_transform(s):
                """V[s%4] (+ shadow slot 4 if s%4==0) from xb[s%6], on GpSimd."""
                src = xb[s % 6]
                slots = [s % 4] + ([4] if s % 4 == 0 else [])
                for cic in range(3):
                    t8 = tp.tile([128, 8, 128], F32, name="btmp", tag="btmp")

                    def xl(l):
                        return src[:, cic, l:l + 512:4]

                    g = nc.vector
                    # E1 = x2 - 4.25 x4 + x6 ; O1 = x1 - 4.25 x3 + x5
                    g.scalar_tensor_tensor(t8[:, 0], xl(4), -4.25, xl(2),
                                           ALU.mult, ALU.add)
                    g.tensor_add(t8[:, 0], t8[:, 0], xl(6))
                    g.scalar_tensor_tensor(t8[:, 1], xl(3), -4.25, xl(1),
                                           ALU.mult, ALU.add)
                    g.tensor_add(t8[:, 1], t8[:, 1], xl(5))
                    # E3 = .25 x2 - 1.25 x4 + x6
                    g.scalar_tensor_tensor(t8[:, 2], xl(4), -1.25, xl(6),
                                           ALU.mult, ALU.add)
                    g.scalar_tensor_tensor(t8[:, 2], xl(2), 0.25, t8[:, 2],
                                           ALU.mult, ALU.add)
                    # O3i = x1 - 5 x3 + 4 x5   (O3 = 0.5*O3i)
                    g.scalar_tensor_tensor(t8[:, 3], xl(3), -5.0, xl(1),
                                           ALU.mult, ALU.add)
                    g.scalar_tensor_tensor(t8[:, 3], xl(5), 4.0, t8[:, 3],
                                           ALU.mult, ALU.add)
                    # E5 = 4 x2 - 5 x4 + x6
                    g.scalar_tensor_tensor(t8[:, 4], xl(2), 4.0, xl(6),
                                           ALU.mult, ALU.add)
                    g.scalar_tensor_tensor(t8[:, 4], xl(4), -5.0, t8[:, 4],
                                           ALU.mult, ALU.add)
                    # O5i = 4 x1 - 5 x3 + x5   (O5 = 0.5*O5i)
                    g.scalar_tensor_tensor(t8[:, 5], xl(1), 4.0, xl(5),
                                           ALU.mult, ALU.add)
                    g.scalar_tensor_tensor(t8[:, 5], xl(3), -5.0, t8[:, 5],
                                           ALU.mult, ALU.add)
                    # V0 = 5.25 (x2 - x4) + (x6 - x0) ; Vinf likewise on odds
                    g.tensor_sub(t8[:, 6], xl(2), xl(4))
                    g.tensor_sub(t8[:, 7], xl(6), xl(0))
                    for sl in slots:
                        v = vt[:, sl, cic]
                        g.scalar_tensor_tensor(v[:, 0], t8[:, 6], 5.25,
                                               t8[:, 7], ALU.mult, ALU.add)
                        # V(+-1) = E1 +- O1
                        g.tensor_add(v[:, 1], t8[:, 0], t8[:, 1])
                        g.tensor_sub(v[:, 2], t8[:, 0], t8[:, 1])
                        # V(+-2) = E3 +- 0.5*O3i   (Bt rows 3,4)
                        g.scalar_tensor_tensor(v[:, 3], t8[:, 3], 0.5,
                                               t8[:, 2], ALU.mult, ALU.add)
                        g.scalar_tensor_tensor(v[:, 4], t8[:, 3], -0.5,
                                               t8[:, 2], ALU.mult, ALU.add)
                        # V(+-1/2) = E5 +- 0.5*O5i (Bt rows 5,6)
                        g.scalar_tensor_tensor(v[:, 5], t8[:, 5], 0.5,
                                               t8[:, 4], ALU.mult, ALU.add)
                        g.scalar_tensor_tensor(v[:, 6], t8[:, 5], -0.5,
                                               t8[:, 4], ALU.mult, ALU.add)
                    # Vinf = -x1 + 5.25 x3 - 5.25 x5 + x7
                    g.tensor_sub(t8[:, 6], xl(3), xl(5))
                    g.tensor_sub(t8[:, 7], xl(7), xl(1))
                    for sl in slots:
                        g.scalar_tensor_tensor(vt[:, sl, cic, 7], t8[:, 6],
                                               5.25, t8[:, 7], ALU.mult,
                                               ALU.add)

            def conv1_pair(j):
                """rows j, j+1 (j even). V rows already produced. Returns the
                two direct-PSUM tiles per coc after wino-A^T accumulation."""
                s = j + 4
                psd_all = []
                for coc in range(3):
                    mps = ppw.tile([128, 8, 2, 128], F32, name="mps", tag="mps")
                    for t in range(8):
                        nmm = 0
                        for kh in (0, 1):
                            s0 = (s + kh - 2) % 4
                            vpair = vt[:, s0:s0 + 2, :, t, :]
                            for cic in range(3):
                                nc.tensor.matmul(
                                    mps[:, t],
                                    lhsT=u1t[:, coc, kh, t, cic, :],
                                    rhs=vpair[:, :, cic, :],
                                    start=(nmm == 0), stop=(nmm == 5))
                                nmm += 1
                    # kh2 direct conv for the two rows
                    psd_rows = []
                    for r in (j, j + 1):
                        sr = (r + 4) % 4
                        psd = ppd.tile([128, W], F32, name="psd", tag="psd")
                        for n8, (kh, kw) in enumerate(FP8_TAPS1):
                            dw = kw - 2
                            fpi = FP8_TAPS1.index((kh, kw))
                            nc.tensor.matmul(
                                psd,
                                lhsT=w18[:, coc, fpi],
                                rhs=x8[sr][:, 0:2, PAD + dw: PAD + dw + W],
                                start=(n8 == 0), stop=False, perf_mode=DRM)
                        work = [(ti, cic) for ti, (kh, kw) in enumerate(DTAPS1)
                                for cic in range(3)
                                if not ((kh, kw) in FP8_TAPS1 and cic < 2)
                                and not (kh == 2 and kw == 2 and cic == 2
                                         and coc == 0)]
                        for n, (ti, cic) in enumerate(work):
                            kh, kw = DTAPS1[ti]
                            dw = kw - 2
                            nc.tensor.matmul(
                                psd,
                                lhsT=w1dt[:, coc, ti, cic, :],
                                rhs=xb[(r + 4) % 6][:, cic,
                                                    PAD + dw: PAD + dw + W],
                                start=False, stop=(n == len(work) - 1))
                        psd_rows.append(psd)
                    # A^T combine on Vector: psd_rows[ri][:, k::4] += sum
                    for ri in (0, 1):
                        psd = psd_rows[ri]
                        wt_ = tp.tile([128, 16, 128], F32, name="atmp",
                                      tag="atmp")
                        v = nc.vector
                        # DVE may read only one PSUM input: stage M in SBUF
                        v.tensor_copy(wt_[:, 0:8], mps[:, :, ri, :])
                        m_ = lambda t: wt_[:, t]
                        w8 = wt_[:, 8:16]
                        v.tensor_add(w8[:, 0], m_(1), m_(2))   # P1
                        v.tensor_sub(w8[:, 1], m_(1), m_(2))   # D1
                        v.tensor_add(w8[:, 2], m_(3), m_(4))   # P2
                        v.tensor_sub(w8[:, 3], m_(3), m_(4))   # D2
                        v.tensor_add(w8[:, 4], m_(5), m_(6))   # P3
                        v.tensor_sub(w8[:, 5], m_(5), m_(6))   # D3
                        pk = lambda k: psd[:, k:512:4]
                        # y0 = M0 + P1 + P2 + P3
                        v.tensor_add(w8[:, 6], m_(0), w8[:, 0])
                        v.tensor_add(w8[:, 6], w8[:, 6], w8[:, 2])
                        v.tensor_add(w8[:, 6], w8[:, 6], w8[:, 4])
                        v.tensor_add(pk(0), pk(0), w8[:, 6])
                        # y1 = D1 + 2 D2 + .5 D3
                        v.scalar_tensor_tensor(w8[:, 6], w8[:, 3], 2.0,
                                               w8[:, 1], ALU.mult, ALU.add)
                        v.scalar_tensor_tensor(w8[:, 6], w8[:, 5], 0.5,
                                               w8[:, 6], ALU.mult, ALU.add)
                        v.tensor_add(pk(1), pk(1), w8[:, 6])
                        # y2 = P1 + 4 P2 + .25 P3
                        v.scalar_tensor_tensor(w8[:, 6], w8[:, 2], 4.0,
                                               w8[:, 0], ALU.mult, ALU.add)
                        v.scalar_tensor_tensor(w8[:, 6], w8[:, 4], 0.25,
                                               w8[:, 6], ALU.mult, ALU.add)
                        v.tensor_add(pk(2), pk(2), w8[:, 6])
                        # y3 = D1 + 8 D2 + .125 D3 + Minf
                        v.scalar_tensor_tensor(w8[:, 6], w8[:, 3], 8.0,
                                               w8[:, 1], ALU.mult, ALU.add)
                        v.scalar_tensor_tensor(w8[:, 6], w8[:, 5], 0.125,
                                               w8[:, 6], ALU.mult, ALU.add)
                        v.tensor_add(w8[:, 6], w8[:, 6], m_(7))
                        v.tensor_add(pk(3), pk(3), w8[:, 6])
                    psd_all.append(psd_rows)
                return psd_all

            def y1_epilogue(psd_all, j):
                for coc in range(3):
                    for ri, r in enumerate((j, j + 1)):
                        slot = (r + 4) % 4
                        ps = psd_all[coc][ri]
                        nc.scalar.activation(
                            y1[slot][:, coc, PAD:PAD + W], ps, AF.Prelu,
                            bias=b1c[:, coc:coc + 1], scale=1.0 / SB,
                            alpha=a1c[:, coc:coc + 1])
                        if coc < 2:
                            nc.scalar.activation(
                                y18[slot][:, coc, PAD:PAD + W], ps, AF.Prelu,
                                bias=b1c16[:, coc:coc + 1], scale=SX / SB,
                                alpha=a1c[:, coc:coc + 1])

            def conv2_row(r, y2s):
                slot = (r + 4) % 4
                for coc in range(3):
                    ps = ppd.tile([128, W], F32, name="ps2", tag="psd")
                    for n8, (kh, kw) in enumerate(FP8_TAPS2):
                        dh, dw = kh - 2, kw - 2
                        src8 = y18[(slot + dh) % 4]
                        fpi = FP8_TAPS2.index((kh, kw))
                        nc.tensor.matmul(
                            ps, lhsT=w28[:, coc, fpi],
                            rhs=src8[:, 0:2, PAD + dw: PAD + dw + W],
                            start=(n8 == 0), stop=False, perf_mode=DRM)
                    taps = sorted(TAPS2, key=lambda t: t[0] == 2)
                    work = [(kh, kw, cic) for (kh, kw) in taps
                            for cic in range(3)
                            if not ((kh, kw) in FP8_TAPS2 and cic < 2)
                            and not (kh == 2 and kw == 2 and cic == 2
                                     and coc == 0)]
                    for n, (kh, kw, cic) in enumerate(work):
                        dh, dw = kh - 2, kw - 2
                        src = y1[(slot + dh) % 4]
                        ti = TAPS2.index((kh, kw))
                        nc.tensor.matmul(
                            ps, lhsT=w2t[:, coc, ti, cic, :],
                            rhs=src[:, cic, PAD + dw: PAD + dw + W],
                            start=False, stop=(n == len(work) - 1))
                    nc.scalar.activation(
                        y2s[:, coc, :], ps, AF.Prelu,
                        bias=b2c[:, coc:coc + 1], scale=1.0 / SB,
                        alpha=a2c[:, coc:coc + 1])
                    nc.vector.tensor_add(
                        y2s[:, coc, :], y2s[:, coc, :],
                        xb[(r + 4) % 6][:, coc, PAD:PAD + W])

            # ---- prologue ----
            for s in range(6):            # x rows -4..1
                load_x_row(s)
            w28 = wp.tile([128, 3, NF2, 2, 128], FP8, name="w28sb", tag="w28sb")
            nc.gpsimd.dma_start(out=w28, in_=w28_d)
            w2t = wp.tile([128, 3, NT2, 3, 128], BF16, name="w2sb", tag="w2sb")
            nc.gpsimd.dma_start(out=w2t, in_=w2_d)
            for s in range(4):            # V for x rows -4..-1
                bt_transform(s)
            cast_x8(2)
            cast_x8(3)
            load_x_row(6)
            load_x_row(7)

            # halo pair: conv1 rows -2,-1 masked by hm
            psd_all = conv1_pair(-2)
            y1_epilogue(psd_all, -2)
            bt_transform(4)
            cast_x8(4)
            bt_transform(5)
            cast_x8(5)
            for hr, r in ((0, -2), (1, -1)):
                slot = (r + 4) % 4
                nc.vector.tensor_scalar_mul(y1[slot], y1[slot],
                                            hm[:, hr:hr + 1])
                for coc in range(2):
                    nc.scalar.activation(
                        y18[slot][:, coc, PAD:PAD + W],
                        y1[slot][:, coc, PAD:PAD + W], AF.Copy, scale=SX)

            # ---- main pair loop ----
            y2s_tiles = {}
            SMAX = 67                     # last xs row actually consumed
            for p in range(NPAIRS):
                j = 2 * p
                if j + 8 <= SMAX:
                    load_x_row(j + 8)
                if j + 9 <= SMAX:
                    load_x_row(j + 9)
                psd_all = conv1_pair(j)
                # B^T for the NEXT pair goes after conv1_pair(j): it
                # overwrites V slots (j+2)%4,(j+3)%4 that pair j still reads
                if j + 6 <= SMAX:
                    bt_transform(j + 6)
                    cast_x8(j + 6)
                if j + 7 <= SMAX:
                    bt_transform(j + 7)
                    cast_x8(j + 7)
                y1_epilogue(psd_all, j)
                for r in (j, j + 1):
                    y2s = op.tile([128, 3, W], BF16, name=f"y2s{r}", tag="y2s")
                    y2s_tiles[r] = y2s
                    conv2_row(r, y2s)
                    if r >= 2:
                        nc.sync.dma_start(out=ys_d[:, ds((r - 2 + 2) * 3, 3), :],
                                          in_=y2s_tiles[r - 2])
            for r in (HS - 2, HS - 1):
                nc.sync.dma_start(out=ys_d[:, ds((r + 2) * 3, 3), :],
                                  in_=y2s_tiles[r])

    nc.compile()
    return nc


_NC_CACHE = {}


def _get_nc():
    if "nc" not in _NC_CACHE:
        _NC_CACHE["nc"] = _build_nc()
    return _NC_CACHE["nc"]


def kernel(x, w1, b1, a1, w2, b2, a2, _trace_dir=None, _trace_cores=None):
    x = np.asarray(x, np.float32)
    mask = _build_mask()
    w1m = np.asarray(w1, np.float32) * mask
    w2m = np.asarray(w2, np.float32) * mask

    # conv1 winograd weights: U[kh][t] = SB * G[t] . w1m[:,:,kh,:]
    # layout [ci_mod(p), coc, kh, t, cic, co_mod]
    u1 = np.empty((128, 3, 2, 8, 3, 128), np.float32)
    for kh in (0, 1):
        gk = w1m[:, :, kh, :].astype(np.float64)          # [co, ci, 5]
        for t in range(8):
            U = (SB * np.tensordot(gk, _G[t], axes=([2], [0]))).astype(
                np.float32)                                # [co, ci]
            Ur = U.reshape(3, 128, 3, 128)                 # [coc, com, cic, cim]
            u1[:, :, kh, t, :, :] = Ur.transpose(3, 0, 2, 1)
    u1_np = np.ascontiguousarray(u1.astype(BF16NP))

    # conv1 direct kh2 taps bf16 (*SB)
    w1d = np.empty((128, 3, 3, 3, 128), np.float32)
    wr1 = (w1m * SB).reshape(3, 128, 3, 128, KS, KS)
    for ti, (kh, kw) in enumerate(DTAPS1):
        w1d[:, :, ti, :, :] = wr1[:, :, :, :, kh, kw].transpose(3, 0, 2, 1)
    w1d_np = np.ascontiguousarray(w1d.astype(BF16NP))

    w18 = np.empty((128, 3, 2, 2, 128), np.float32)
    wr18 = (w1m * SW).reshape(3, 128, 3, 128, KS, KS)
    for ti, (kh, kw) in enumerate(FP8_TAPS1):
        w18[:, :, ti, :, :] = wr18[:, :, 0:2, :, kh, kw].transpose(3, 0, 2, 1)
    w18_np = np.ascontiguousarray(w18.astype(E4NP))

    # conv2 weights (baseline layouts)
    def wT(wm):
        wr = (wm * SB).reshape(3, 128, 3, 128, KS, KS)
        out = np.empty((128, 3, NT2, 3, 128), np.float32)
        for t, (kh, kw) in enumerate(TAPS2):
            out[:, :, t, :, :] = wr[:, :, :, :, kh, kw].transpose(3, 0, 2, 1)
        return np.ascontiguousarray(out.astype(BF16NP))

    def wT8(wm):
        wr = (wm * SW).reshape(3, 128, 3, 128, KS, KS)
        out = np.empty((128, 3, NF2, 2, 128), np.float32)
        for t, (kh, kw) in enumerate(FP8_TAPS2):
            out[:, :, t, :, :] = wr[:, :, 0:2, :, kh, kw].transpose(3, 0, 2, 1)
        return np.ascontiguousarray(out.astype(E4NP))

    w2t_np, w28_np = wT(w2m), wT8(w2m)

    def chunked(v):
        return np.ascontiguousarray(np.asarray(v, np.float32).reshape(3, 128).T)

    b1c, a1c = chunked(b1), chunked(a1)
    b2c, a2c = chunked(b2), chunked(a2)
    b1c16 = np.ascontiguousarray(b1c * np.float32(SX))

    xq = x.reshape(B, 3, 128, H, W)
    in_maps = []
    for core in range(NCORES):
        b_, s = divmod(core, SPB)
        r0 = s * HS
        xs = np.zeros((128, NR, 3, WP), BF16NP)
        lo, hi = r0 - 4, r0 - 4 + NR
        glo, ghi = max(lo, 0), min(hi, H)
        if ghi > glo:
            xs[:, glo - lo:ghi - lo, :, PAD:PAD + W] = \
                xq[b_, :, :, glo:ghi, :].transpose(1, 2, 0, 3)
        hmv = (np.zeros((128, 2), np.float32) if s == 0
               else np.ones((128, 2), np.float32))
        in_maps.append({
            "xs": xs.reshape(128, NR * 3, WP),
            "u1t": u1_np, "w1d": w1d_np, "w18": w18_np,
            "w2t": w2t_np, "w28": w28_np,
            "b1c": b1c, "a1c": a1c, "b2c": b2c, "a2c": a2c,
            "b1c16": b1c16, "hm": hmv,
        })

    nc = _get_nc()
    kw = {}
    if _trace_dir is not None:
        kw = dict(trace=True, tmpdir=_trace_dir,
                  trace_cores=_trace_cores or [0])

    def gather(res):
        y = np.empty_like(x)
        for core in range(NCORES):
            b_, s = divmod(core, SPB)
            r0 = s * HS
            ys = res.results[core]["ys"].reshape(128, HS + 2, 3, W)[:, 2:]
            y[b_, :, r0:r0 + HS, :] = \
                ys.transpose(2, 0, 1, 3).reshape(C, HS, W).astype(np.float32)
        return y

    res = y = None
    for attempt in range(4):
        try:
            res = run_bass_kernel_spmd(nc, in_maps,
                                       core_ids=list(range(NCORES)), **kw)
            y = gather(res)
            if np.isfinite(y).all() and np.abs(y).max() < 50.0:
                break
            if attempt == 3:
                break
        except Exception:
            if attempt == 3:
                raise
            import time
            time.sleep(5)

    if _trace_dir is not None:
        return y, res
    return y


# revision 3
# speedup vs baseline: 1.0173x; 1.0173x over previous
"""Trainium2 Bass kernel for the EntropyResidualBlock — Winograd variant.

conv1's kh=0,1 tap rows (10 of 13 taps) run as Winograd F(4,5) along W:
B^T input transform on GpSimd (from the bf16 x ring), 8 point-matmuls per
(kh,cic,coc) batched over 2 output rows (N=256, bf16 U/V), A^T output
combine on Vector accumulating into the kh=2 direct-conv PSUM. conv1 kh=2
taps: (2,0),(2,1) cic01 as fp8e4 DoubleRow, rest bf16. conv2 is the
baseline direct conv (5 fp8-DR taps + bf16). Offline sim rel err 1.755e-2.

Sharding: 8 cores = 2 batches x 4 H-strips of 64 rows (as baseline), with a
2-row recomputed y1 halo masked by hm for top strips.
"""

import os
import sys

import numpy as np
import ml_dtypes

for _p in ("/opt/trn_rl_repo",):
    if os.path.isdir(_p) and _p not in sys.path:
        sys.path.append(_p)

import concourse.bass as bass  # noqa: E402
import concourse.tile as tile  # noqa: E402
from concourse import bacc, mybir  # noqa: E402
from concourse.bass import ds  # noqa: E402
from concourse.bass_utils import run_bass_kernel_spmd  # noqa: E402

BF16NP = ml_dtypes.bfloat16
E4NP = ml_dtypes.float8_e4m3
F32 = mybir.dt.float32
BF16 = mybir.dt.bfloat16
FP8 = mybir.dt.float8e4
AF = mybir.ActivationFunctionType
DRM = mybir.MatmulPerfMode.DoubleRow
ALU = mybir.AluOpType

B, C, H, W = 2, 384, 256, 512
NG, CPN, KS, PAD = 16, 24, 5, 2
NCORES = 8
SPB = 4            # strips per batch
HS = H // SPB      # 64 output rows per core
WP = 520           # padded row width (2 left + 512 + 6 right, all zeros)
WP8 = 528          # fp8 ring row pitch
NR = HS + 5        # x rows staged per core
NT2 = 13           # conv2 direct taps
TAPS2 = [(kh, kw) for kh in (0, 1) for kw in range(KS)] + [(2, 0), (2, 1), (2, 2)]
FP8_TAPS2 = [(0, 0), (0, 2), (0, 4), (1, 1), (1, 3), (2, 2)]
NF2 = len(FP8_TAPS2)
DTAPS1 = [(2, 0), (2, 1), (2, 2)]      # conv1 direct taps
FP8_TAPS1 = [(2, 0), (2, 1), (2, 2)]
NF1 = len(FP8_TAPS1)
SW, SX = 1024.0, 16.0
SB = SW * SX
NPAIRS = HS // 2


def _wino_mats():
    """F(4,5) Cook-Toom at points [0,1,-1,2,-2,1/2,-1/2,inf]; exact (verified
    to 1e-13 by construction in the offline sim)."""
    m, R = 4, 5
    pts = [0, 1, -1, 2, -2, 0.5, -0.5]
    n = m + R - 1
    a = np.array(pts, np.float64)
    At = np.zeros((m, n))
    for i in range(m):
        At[i, :n - 1] = a ** i
    At[m - 1, n - 1] = 1.0
    G = np.zeros((n, R))
    for i in range(n - 1):
        Ni = np.prod([a[i] - a[k] for k in range(n - 1) if k != i])
        G[i] = (a[i] ** np.arange(R)) / Ni
    G[n - 1, R - 1] = 1.0
    M = np.zeros((m * R, n))
    for k in range(m):
        for j in range(R):
            M[k * R + j] = At[k] * G[:, j]
    Bt = np.zeros((n, n))
    for l in range(n):
        c = np.zeros(m * R)
        for k in range(m):
            for j in range(R):
                c[k * R + j] = 1.0 if l == k + j else 0.0
        Bt[:, l] = np.linalg.lstsq(M, c, rcond=None)[0]
    return At, G, Bt


_AT, _G, _BT = _wino_mats()


def _build_mask() -> np.ndarray:
    m = np.zeros((C, C, KS, KS), np.float32)
    m[:, :, :PAD, :] = 1.0
    m[:, :, PAD, :PAD] = 1.0
    g = np.arange(C) // CPN
    m[:, :, PAD, PAD] = (g[None, :] <= g[:, None]).astype(np.float32)
    return m


def _build_nc():
    nc = bacc.Bacc("TRN2", target_bir_lowering=False, debug=False,
                   num_devices=NCORES)
    xs_d = nc.dram_tensor("xs", [128, NR * 3, WP], BF16, kind="ExternalInput").ap()
    u1_d = nc.dram_tensor("u1t", [128, 3, 2, 8, 3, 128], BF16,
                          kind="ExternalInput").ap()
    w1d_d = nc.dram_tensor("w1d", [128, 3, 3, 3, 128], BF16,
                           kind="ExternalInput").ap()
    w18_d = nc.dram_tensor("w18", [128, 3, NF1, 2, 128], FP8,
                           kind="ExternalInput").ap()
    w2_d = nc.dram_tensor("w2t", [128, 3, NT2, 3, 128], BF16,
                          kind="ExternalInput").ap()
    w28_d = nc.dram_tensor("w28", [128, 3, NF2, 2, 128], FP8,
                           kind="ExternalInput").ap()
    b1_d = nc.dram_tensor("b1c", [128, 3], F32, kind="ExternalInput").ap()
    a1_d = nc.dram_tensor("a1c", [128, 3], F32, kind="ExternalInput").ap()
    b2_d = nc.dram_tensor("b2c", [128, 3], F32, kind="ExternalInput").ap()
    a2_d = nc.dram_tensor("a2c", [128, 3], F32, kind="ExternalInput").ap()
    b116_d = nc.dram_tensor("b1c16", [128, 3], F32, kind="ExternalInput").ap()
    hm_d = nc.dram_tensor("hm", [128, 2], F32, kind="ExternalInput").ap()
    ys_d = nc.dram_tensor("ys", [128, (HS + 2) * 3, W], BF16,
                          kind="ExternalOutput").ap()

    with tile.TileContext(nc) as tc:
        with tc.tile_pool(name="wp", bufs=1) as wp, \
             tc.tile_pool(name="cp", bufs=1) as cp, \
             tc.tile_pool(name="ring", bufs=1) as rp, \
             tc.tile_pool(name="tmp", bufs=3) as tp, \
             tc.tile_pool(name="op", bufs=4) as op, \
             tc.tile_pool(name="ppw", bufs=1, space="PSUM") as ppw, \
             tc.tile_pool(name="ppd", bufs=4, space="PSUM") as ppd:

            w18 = wp.tile([128, 3, NF1, 2, 128], FP8, name="w18sb", tag="w18sb")
            nc.gpsimd.dma_start(out=w18, in_=w18_d)
            u1t = wp.tile([128, 3, 2, 8, 3, 128], BF16, name="u1sb", tag="u1sb")
            for _c in range(3):
                nc.gpsimd.dma_start(out=u1t[:, _c], in_=u1_d[:, _c])
            w1dt = wp.tile([128, 3, 3, 3, 128], BF16, name="w1dsb", tag="w1dsb")
            nc.gpsimd.dma_start(out=w1dt, in_=w1d_d)
            b1c = cp.tile([128, 3], F32, name="b1sb", tag="b1sb")
            nc.gpsimd.dma_start(out=b1c, in_=b1_d)
            a1c = cp.tile([128, 3], F32, name="a1sb", tag="a1sb")
            nc.gpsimd.dma_start(out=a1c, in_=a1_d)
            b2c = cp.tile([128, 3], F32, name="b2sb", tag="b2sb")
            nc.gpsimd.dma_start(out=b2c, in_=b2_d)
            a2c = cp.tile([128, 3], F32, name="a2sb", tag="a2sb")
            nc.gpsimd.dma_start(out=a2c, in_=a2_d)
            b1c16 = cp.tile([128, 3], F32, name="b116sb", tag="b116sb")
            nc.gpsimd.dma_start(out=b1c16, in_=b116_d)
            hm = cp.tile([128, 2], F32, name="hmsb", tag="hmsb")
            nc.gpsimd.dma_start(out=hm, in_=hm_d)

            # rings: xb keyed s%6 (s = x row + 4), x8 and y-rings keyed %4,
            # V keyed s%4 with slot 4 = shadow of slot 0
            xb = [rp.tile([128, 3, WP], BF16, name=f"xb{j}", tag=f"xb{j}")
                  for j in range(6)]
            x8 = [rp.tile([128, 2, WP8], FP8, name=f"x8{j}", tag=f"x8{j}")
                  for j in range(4)]
            vt = rp.tile([128, 5, 3, 8, 128], BF16, name="vt", tag="vt")
            y1 = [rp.tile([128, 3, WP], BF16, name=f"y1r{j}", tag=f"y1r{j}")
                  for j in range(4)]
            y18 = [rp.tile([128, 2, WP8], FP8, name=f"y18r{j}", tag=f"y18r{j}")
                   for j in range(4)]
            for j in range(4):
                nc.vector.memset(y1[j][:, :, 0:PAD], 0.0)
                nc.vector.memset(y1[j][:, :, PAD + W:WP], 0.0)
                nc.vector.memset(y18[j][:, :, 0:PAD], 0.0)
                nc.vector.memset(y18[j][:, :, PAD + W:WP8], 0.0)

            def load_x_row(s):
                # xs row s (= x row s-4): one DMA
                nc.sync.dma_start(out=xb[s % 6], in_=xs_d[:, ds(s * 3, 3), :])

            def cast_x8(s):
                nc.scalar.activation(x8[s % 4][:, :, 0:WP],
                                     xb[s % 6][:, 0:2, :], AF.Copy, scale=SX)

            def bt_transform(s):
                """V[s%4] (+ shadow slot 4 if s%4==0) from xb[s%6], on GpSimd."""
                src = xb[s % 6]
                slots = [s % 4] + ([4] if s % 4 == 0 else [])
                for cic in range(3):
                    t8 = tp.tile([128, 8, 128], F32, name="btmp", tag="btmp")

                    def xl(l):
                        return src[:, cic, l:l + 512:4]

                    g = nc.vector
                    # E1 = x2 - 4.25 x4 + x6 ; O1 = x1 - 4.25 x3 + x5
                    g.scalar_tensor_tensor(t8[:, 0], xl(4), -4.25, xl(2),
                                           ALU.mult, ALU.add)
                    g.tensor_add(t8[:, 0], t8[:, 0], xl(6))
                    g.scalar_tensor_tensor(t8[:, 1], xl(3), -4.25, xl(1),
                                           ALU.mult, ALU.add)
                    g.tensor_add(t8[:, 1], t8[:, 1], xl(5))
                    # E3 = .25 x2 - 1.25 x4 + x6
                    g.scalar_tensor_tensor(t8[:, 2], xl(4), -1.25, xl(6),
                                           ALU.mult, ALU.add)
                    g.scalar_tensor_tensor(t8[:, 2], xl(2), 0.25, t8[:, 2],
                                           ALU.mult, ALU.add)
                    # O3i = x1 - 5 x3 + 4 x5   (O3 = 0.5*O3i)
                    g.scalar_tensor_tensor(t8[:, 3], xl(3), -5.0, xl(1),
                                           ALU.mult, ALU.add)
                    g.scalar_tensor_tensor(t8[:, 3], xl(5), 4.0, t8[:, 3],
                                           ALU.mult, ALU.add)
                    # E5 = 4 x2 - 5 x4 + x6
                    g.scalar_tensor_tensor(t8[:, 4], xl(2), 4.0, xl(6),
                                           ALU.mult, ALU.add)
                    g.scalar_tensor_tensor(t8[:, 4], xl(4), -5.0, t8[:, 4],
                                           ALU.mult, ALU.add)
                    # O5i = 4 x1 - 5 x3 + x5   (O5 = 0.5*O5i)
                    g.scalar_tensor_tensor(t8[:, 5], xl(1), 4.0, xl(5),
                                           ALU.mult, ALU.add)
                    g.scalar_tensor_tensor(t8[:, 5], xl(3), -5.0, t8[:, 5],
                                           ALU.mult, ALU.add)
                    # V0 = 5.25 (x2 - x4) + (x6 - x0) ; Vinf likewise on odds
                    g.tensor_sub(t8[:, 6], xl(2), xl(4))
                    g.tensor_sub(t8[:, 7], xl(6), xl(0))
                    for sl in slots:
                        v = vt[:, sl, cic]
                        g.scalar_tensor_tensor(v[:, 0], t8[:, 6], 5.25,
                                               t8[:, 7], ALU.mult, ALU.add)
                        # V(+-1) = E1 +- O1
                        g.tensor_add(v[:, 1], t8[:, 0], t8[:, 1])
                        g.tensor_sub(v[:, 2], t8[:, 0], t8[:, 1])
                        # V(+-2) = E3 +- 0.5*O3i   (Bt rows 3,4)
                        g.scalar_tensor_tensor(v[:, 3], t8[:, 3], 0.5,
                                               t8[:, 2], ALU.mult, ALU.add)
                        g.scalar_tensor_tensor(v[:, 4], t8[:, 3], -0.5,
                                               t8[:, 2], ALU.mult, ALU.add)
                        # V(+-1/2) = E5 +- 0.5*O5i (Bt rows 5,6)
                        g.scalar_tensor_tensor(v[:, 5], t8[:, 5], 0.5,
                                               t8[:, 4], ALU.mult, ALU.add)
                        g.scalar_tensor_tensor(v[:, 6], t8[:, 5], -0.5,
                                               t8[:, 4], ALU.mult, ALU.add)
                    # Vinf = -x1 + 5.25 x3 - 5.25 x5 + x7
                    g.tensor_sub(t8[:, 6], xl(3), xl(5))
                    g.tensor_sub(t8[:, 7], xl(7), xl(1))
                    for sl in slots:
                        g.scalar_tensor_tensor(vt[:, sl, cic, 7], t8[:, 6],
                                               5.25, t8[:, 7], ALU.mult,
                                               ALU.add)

            def conv1_pair(j):
                """rows j, j+1 (j even). V rows already produced. Returns the
                two direct-PSUM tiles per coc after wino-A^T accumulation."""
                s = j + 4
                psd_all = []
                for coc in range(3):
                    mps = ppw.tile([128, 8, 2, 128], F32, name="mps", tag="mps")
                    for t in range(8):
                        nmm = 0
                        for kh in (0, 1):
                            s0 = (s + kh - 2) % 4
                            vpair = vt[:, s0:s0 + 2, :, t, :]
                            for cic in range(3):
                                nc.tensor.matmul(
                                    mps[:, t],
                                    lhsT=u1t[:, coc, kh, t, cic, :],
                                    rhs=vpair[:, :, cic, :],
                                    start=(nmm == 0), stop=(nmm == 5))
                                nmm += 1
                    # A^T part 1 (Vector, mps-only): stage M, compute the
                    # four combined rows into wt_ so the single mps buffer is
                    # released while the kh2 MMs run.
                    part1 = []
                    for ri in (0, 1):
                        wt_ = tp.tile([128, 16, 128], F32, name="atmp",
                                      tag="atmp")
                        part1.append(wt_)
                        v = nc.vector
                        v.tensor_copy(wt_[:, 0:8], mps[:, :, ri, :])
                        m_ = lambda t: wt_[:, t]
                        w8 = wt_[:, 8:16]
                        v.tensor_add(w8[:, 0], m_(1), m_(2))   # P1
                        v.tensor_sub(w8[:, 1], m_(1), m_(2))   # D1
                        v.tensor_add(w8[:, 2], m_(3), m_(4))   # P2
                        v.tensor_sub(w8[:, 3], m_(3), m_(4))   # D2
                        v.tensor_add(w8[:, 4], m_(5), m_(6))   # P3
                        v.tensor_sub(w8[:, 5], m_(5), m_(6))   # D3
                        # y0 -> wt_[:,1]
                        v.tensor_add(wt_[:, 1], m_(0), w8[:, 0])
                        v.tensor_add(wt_[:, 1], wt_[:, 1], w8[:, 2])
                        v.tensor_add(wt_[:, 1], wt_[:, 1], w8[:, 4])
                        # y1 -> wt_[:,2]
                        v.scalar_tensor_tensor(wt_[:, 2], w8[:, 3], 2.0,
                                               w8[:, 1], ALU.mult, ALU.add)
                        v.scalar_tensor_tensor(wt_[:, 2], w8[:, 5], 0.5,
                                               wt_[:, 2], ALU.mult, ALU.add)
                        # y2 -> wt_[:,3]
                        v.scalar_tensor_tensor(wt_[:, 3], w8[:, 2], 4.0,
                                               w8[:, 0], ALU.mult, ALU.add)
                        v.scalar_tensor_tensor(wt_[:, 3], w8[:, 4], 0.25,
                                               wt_[:, 3], ALU.mult, ALU.add)
                        # y3 -> wt_[:,4]
                        v.scalar_tensor_tensor(wt_[:, 4], w8[:, 3], 8.0,
                                               w8[:, 1], ALU.mult, ALU.add)
                        v.scalar_tensor_tensor(wt_[:, 4], w8[:, 5], 0.125,
                                               wt_[:, 4], ALU.mult, ALU.add)
                        v.tensor_add(wt_[:, 4], wt_[:, 4], m_(7))
                    # kh2 direct conv for the two rows
                    psd_rows = []
                    for r in (j, j + 1):
                        sr = (r + 4) % 4
                        psd = ppd.tile([128, W], F32, name="psd", tag="psd")
                        for n8, (kh, kw) in enumerate(FP8_TAPS1):
                            dw = kw - 2
                            fpi = FP8_TAPS1.index((kh, kw))
                            nc.tensor.matmul(
                                psd,
                                lhsT=w18[:, coc, fpi],
                                rhs=x8[sr][:, 0:2, PAD + dw: PAD + dw + W],
                                start=(n8 == 0), stop=False, perf_mode=DRM)
                        work = [(ti, cic) for ti, (kh, kw) in enumerate(DTAPS1)
                                for cic in range(3)
                                if not ((kh, kw) in FP8_TAPS1 and cic < 2)
                                and not (kh == 2 and kw == 2 and cic == 2
                                         and coc == 0)]
                        for n, (ti, cic) in enumerate(work):
                            kh, kw = DTAPS1[ti]
                            dw = kw - 2
                            nc.tensor.matmul(
                                psd,
                                lhsT=w1dt[:, coc, ti, cic, :],
                                rhs=xb[(r + 4) % 6][:, cic,
                                                    PAD + dw: PAD + dw + W],
                                start=False, stop=(n == len(work) - 1))
                        psd_rows.append(psd)
                    # A^T part 2 (Vector): accumulate into psd
                    for ri in (0, 1):
                        psd = psd_rows[ri]
                        wt_ = part1[ri]
                        v = nc.vector
                        pk = lambda k: psd[:, k:512:4]
                        for k in range(4):
                            v.tensor_add(pk(k), pk(k), wt_[:, 1 + k])
                    psd_all.append(psd_rows)
                return psd_all

            def y1_epilogue(psd_all, j):
                for coc in range(3):
                    for ri, r in enumerate((j, j + 1)):
                        slot = (r + 4) % 4
                        ps = psd_all[coc][ri]
                        nc.scalar.activation(
                            y1[slot][:, coc, PAD:PAD + W], ps, AF.Prelu,
                            bias=b1c[:, coc:coc + 1], scale=1.0 / SB,
                            alpha=a1c[:, coc:coc + 1])
                        if coc < 2:
                            nc.scalar.activation(
                                y18[slot][:, coc, PAD:PAD + W], ps, AF.Prelu,
                                bias=b1c16[:, coc:coc + 1], scale=SX / SB,
                                alpha=a1c[:, coc:coc + 1])

            def conv2_row(r, y2s):
                slot = (r + 4) % 4
                for coc in range(3):
                    ps = ppd.tile([128, W], F32, name="ps2", tag="psd")
                    for n8, (kh, kw) in enumerate(FP8_TAPS2):
                        dh, dw = kh - 2, kw - 2
                        src8 = y18[(slot + dh) % 4]
                        fpi = FP8_TAPS2.index((kh, kw))
                        nc.tensor.matmul(
                            ps, lhsT=w28[:, coc, fpi],
                            rhs=src8[:, 0:2, PAD + dw: PAD + dw + W],
                            start=(n8 == 0), stop=False, perf_mode=DRM)
                    taps = sorted(TAPS2, key=lambda t: t[0] == 2)
                    work = [(kh, kw, cic) for (kh, kw) in taps
                            for cic in range(3)
                            if not ((kh, kw) in FP8_TAPS2 and cic < 2)
                            and not (kh == 2 and kw == 2 and cic == 2
                                     and coc == 0)]
                    for n, (kh, kw, cic) in enumerate(work):
                        dh, dw = kh - 2, kw - 2
                        src = y1[(slot + dh) % 4]
                        ti = TAPS2.index((kh, kw))
                        nc.tensor.matmul(
                            ps, lhsT=w2t[:, coc, ti, cic, :],
                            rhs=src[:, cic, PAD + dw: PAD + dw + W],
                            start=False, stop=(n == len(work) - 1))
                    nc.scalar.activation(
                        y2s[:, coc, :], ps, AF.Prelu,
                        bias=b2c[:, coc:coc + 1], scale=1.0 / SB,
                        alpha=a2c[:, coc:coc + 1])
                    nc.vector.tensor_add(
                        y2s[:, coc, :], y2s[:, coc, :],
                        xb[(r + 4) % 6][:, coc, PAD:PAD + W])

            # ---- prologue ----
            for s in range(6):            # x rows -4..1
                load_x_row(s)
            w28 = wp.tile([128, 3, NF2, 2, 128], FP8, name="w28sb", tag="w28sb")
            nc.gpsimd.dma_start(out=w28, in_=w28_d)
            w2t = wp.tile([128, 3, NT2, 3, 128], BF16, name="w2sb", tag="w2sb")
            nc.gpsimd.dma_start(out=w2t, in_=w2_d)
            for s in range(4):            # V for x rows -4..-1
                bt_transform(s)
            cast_x8(2)
            cast_x8(3)
            load_x_row(6)
            load_x_row(7)

            # halo pair: conv1 rows -2,-1 masked by hm
            psd_all = conv1_pair(-2)
            y1_epilogue(psd_all, -2)
            bt_transform(4)
            cast_x8(4)
            bt_transform(5)
            cast_x8(5)
            for hr, r in ((0, -2), (1, -1)):
                slot = (r + 4) % 4
                nc.vector.tensor_scalar_mul(y1[slot], y1[slot],
                                            hm[:, hr:hr + 1])
                for coc in range(2):
                    nc.scalar.activation(
                        y18[slot][:, coc, PAD:PAD + W],
                        y1[slot][:, coc, PAD:PAD + W], AF.Copy, scale=SX)

            # ---- main pair loop ----
            y2s_tiles = {}
            SMAX = 67                     # last xs row actually consumed
            for p in range(NPAIRS):
                j = 2 * p
                if j + 8 <= SMAX:
                    load_x_row(j + 8)
                if j + 9 <= SMAX:
                    load_x_row(j + 9)
                psd_all = conv1_pair(j)
                # B^T for the NEXT pair goes after conv1_pair(j): it
                # overwrites V slots (j+2)%4,(j+3)%4 that pair j still reads
                if j + 6 <= SMAX:
                    bt_transform(j + 6)
                    cast_x8(j + 6)
                if j + 7 <= SMAX:
                    bt_transform(j + 7)
                    cast_x8(j + 7)
                y1_epilogue(psd_all, j)
                for r in (j, j + 1):
                    y2s = op.tile([128, 3, W], BF16, name=f"y2s{r}", tag="y2s")
                    y2s_tiles[r] = y2s
                    conv2_row(r, y2s)
                    if r >= 2:
                        nc.sync.dma_start(out=ys_d[:, ds((r - 2 + 2) * 3, 3), :],
                                          in_=y2s_tiles[r - 2])
            for r in (HS - 2, HS - 1):
                nc.sync.dma_start(out=ys_d[:, ds((r + 2) * 3, 3), :],
                                  in_=y2s_tiles[r])

    nc.compile()
    return nc


_NC_CACHE = {}


def _get_nc():
    if "nc" not in _NC_CACHE:
        _NC_CACHE["nc"] = _build_nc()
    return _NC_CACHE["nc"]


def kernel(x, w1, b1, a1, w2, b2, a2, _trace_dir=None, _trace_cores=None):
    x = np.asarray(x, np.float32)
    mask = _build_mask()
    w1m = np.asarray(w1, np.float32) * mask
    w2m = np.asarray(w2, np.float32) * mask

    # conv1 winograd weights: U[kh][t] = SB * G[t] . w1m[:,:,kh,:]
    # layout [ci_mod(p), coc, kh, t, cic, co_mod]
    u1 = np.empty((128, 3, 2, 8, 3, 128), np.float32)
    for kh in (0, 1):
        gk = w1m[:, :, kh, :].astype(np.float64)          # [co, ci, 5]
        for t in range(8):
            U = (SB * np.tensordot(gk, _G[t], axes=([2], [0]))).astype(
                np.float32)                                # [co, ci]
            Ur = U.reshape(3, 128, 3, 128)                 # [coc, com, cic, cim]
            u1[:, :, kh, t, :, :] = Ur.transpose(3, 0, 2, 1)
    u1_np = np.ascontiguousarray(u1.astype(BF16NP))

    # conv1 direct kh2 taps bf16 (*SB)
    w1d = np.empty((128, 3, 3, 3, 128), np.float32)
    wr1 = (w1m * SB).reshape(3, 128, 3, 128, KS, KS)
    for ti, (kh, kw) in enumerate(DTAPS1):
        w1d[:, :, ti, :, :] = wr1[:, :, :, :, kh, kw].transpose(3, 0, 2, 1)
    w1d_np = np.ascontiguousarray(w1d.astype(BF16NP))

    w18 = np.empty((128, 3, NF1, 2, 128), np.float32)
    wr18 = (w1m * SW).reshape(3, 128, 3, 128, KS, KS)
    for ti, (kh, kw) in enumerate(FP8_TAPS1):
        w18[:, :, ti, :, :] = wr18[:, :, 0:2, :, kh, kw].transpose(3, 0, 2, 1)
    w18_np = np.ascontiguousarray(w18.astype(E4NP))

    # conv2 weights (baseline layouts)
    def wT(wm):
        wr = (wm * SB).reshape(3, 128, 3, 128, KS, KS)
        out = np.empty((128, 3, NT2, 3, 128), np.float32)
        for t, (kh, kw) in enumerate(TAPS2):
            out[:, :, t, :, :] = wr[:, :, :, :, kh, kw].transpose(3, 0, 2, 1)
        return np.ascontiguousarray(out.astype(BF16NP))

    def wT8(wm):
        wr = (wm * SW).reshape(3, 128, 3, 128, KS, KS)
        out = np.empty((128, 3, NF2, 2, 128), np.float32)
        for t, (kh, kw) in enumerate(FP8_TAPS2):
            out[:, :, t, :, :] = wr[:, :, 0:2, :, kh, kw].transpose(3, 0, 2, 1)
        return np.ascontiguousarray(out.astype(E4NP))

    w2t_np, w28_np = wT(w2m), wT8(w2m)

    def chunked(v):
        return np.ascontiguousarray(np.asarray(v, np.float32).reshape(3, 128).T)

    b1c, a1c = chunked(b1), chunked(a1)
    b2c, a2c = chunked(b2), chunked(a2)
    b1c16 = np.ascontiguousarray(b1c * np.float32(SX))

    xq = x.reshape(B, 3, 128, H, W)
    in_maps = []
    for core in range(NCORES):
        b_, s = divmod(core, SPB)
        r0 = s * HS
        xs = np.zeros((128, NR, 3, WP), BF16NP)
        lo, hi = r0 - 4, r0 - 4 + NR
        glo, ghi = max(lo, 0), min(hi, H)
        if ghi > glo:
            xs[:, glo - lo:ghi - lo, :, PAD:PAD + W] = \
                xq[b_, :, :, glo:ghi, :].transpose(1, 2, 0, 3)
        hmv = (np.zeros((128, 2), np.float32) if s == 0
               else np.ones((128, 2), np.float32))
        in_maps.append({
            "xs": xs.reshape(128, NR * 3, WP),
            "u1t": u1_np, "w1d": w1d_np, "w18": w18_np,
            "w2t": w2t_np, "w28": w28_np,
            "b1c": b1c, "a1c": a1c, "b2c": b2c, "a2c": a2c,
            "b1c16": b1c16, "hm": hmv,
        })

    nc = _get_nc()
    kw = {}
    if _trace_dir is not None:
        kw = dict(trace=True, tmpdir=_trace_dir,
                  trace_cores=_trace_cores or [0])

    def gather(res):
        y = np.empty_like(x)
        for core in range(NCORES):
            b_, s = divmod(core, SPB)
            r0 = s * HS
            ys = res.results[core]["ys"].reshape(128, HS + 2, 3, W)[:, 2:]
            y[b_, :, r0:r0 + HS, :] = \
                ys.transpose(2, 0, 1, 3).reshape(C, HS, W).astype(np.float32)
        return y

    res = y = None
    for attempt in range(4):
        try:
            res = run_bass_kernel_spmd(nc, in_maps,
                                       core_ids=list(range(NCORES)), **kw)
            y = gather(res)
            if np.isfinite(y).all() and np.abs(y).max() < 50.0:
                break
            if attempt == 3:
                break
        except Exception:
            if attempt == 3:
                raise
            import time
            time.sleep(5)

    if _trace_dir is not None:
        return y, res
    return y
